# revision 4
# baseline (speedup 1.0000x reference)
"""Causal self-attention (B=4, T=2048, C=2048, H=16) on 8 trn2 NeuronCores.

Sharding: tensor-parallel over heads — 2 heads per core. Every core gets the
full (pre-transposed) activation xT, its 2 heads' slice of Wqkv columns and
Wproj rows, computes a full [B*T, C] partial output (fp16), and the host sums
the 8 partials (the "all-reduce after output projection" done host-side).

Per-core dataflow (all matmuls fp16 on PE):
  xT tiles --DMA--> QKV proj -> Q^T,K^T [d,t] + V [t,d]
  S = Q^T.T @ K^T chunks (PSUM f32) -> +causal mask -> exp (ACT) -> P (fp16)
  y^T = sum_k V_k^T-block @ P^T-block (PSUM f32, accumulated over k-blocks)
  softmax denominator: DVE reduce of P over k-blocks -> one ones-matmul ->
  reciprocal_approx_fast -> PE row-broadcast -> DVE normalize.
  The den/rec/normalize epilogue is software-pipelined two attention units
  deep so the in-order PE queue never waits on the DVE/ACT chain (a PE stall
  also drops the PE to its half-speed p-state for ~3us).
  out_partial = y^T.T @ Wproj-rows (accumulate 2 head-chunks) -> fp16 -> DMA
"""
import numpy as np

B, T, C = 4, 2048, 2048
H, HD = 16, 128
N_CORES = 8
HPC = H // N_CORES          # heads per core = 2
SCALE = float(1.0 / np.sqrt(HD))
NEG = -1e9

_CACHE = {}


def _build_nc():
    import concourse.bass as bass
    from concourse import bacc
    import concourse.tile as tile
    import concourse.mybir as mybir
    from concourse.masks import make_identity
    from contextlib import ExitStack

    f32 = mybir.dt.float32
    f16 = mybir.dt.float16
    Exp = mybir.ActivationFunctionType.Exp
    AXX = mybir.AxisListType.X
    Add = mybir.AluOpType.add

    nc = bacc.Bacc("TRN2", target_bir_lowering=False, debug=False,
                   enable_asserts=True, num_devices=N_CORES)

    # Inputs (per-core shards prepared on host)
    xT = nc.dram_tensor("xt", [C, B * T], f16, kind="ExternalInput").ap()
    wqkv = nc.dram_tensor("wqkv", [C, 6 * HD], f16, kind="ExternalInput").ap()
    wproj = nc.dram_tensor("wproj", [HPC * HD, C], f16, kind="ExternalInput").ap()
    out = nc.dram_tensor("out", [B * T, C], f16, kind="ExternalOutput").ap()

    # DRAM views: c-chunked weights
    wqkv_v = wqkv.rearrange("(cc p) (jj d) -> p cc jj d", p=128, d=HD)  # [128,16,6,128]
    wproj_v = wproj.rearrange("(jh p) c -> p jh c", p=128)              # [128,2,2048]

    NCC = C // 128        # 16 contraction chunks
    NTCH = T // 512       # 4 t-chunks per batch

    with tile.TileContext(nc) as tc, ExitStack() as ctx:
        const = ctx.enter_context(tc.tile_pool(name="const", bufs=1))
        wpool = ctx.enter_context(tc.tile_pool(name="w", bufs=1))
        xtp = ctx.enter_context(tc.tile_pool(name="xt", bufs=2))
        qkvp = ctx.enter_context(tc.tile_pool(name="qkv", bufs=2))

        dnp = ctx.enter_context(tc.tile_pool(name="dn", bufs=2))
        rp = ctx.enter_context(tc.tile_pool(name="r", bufs=2))
        ptp = ctx.enter_context(tc.tile_pool(name="pt", bufs=2))
        ytp = ctx.enter_context(tc.tile_pool(name="yt", bufs=2))
        op = ctx.enter_context(tc.tile_pool(name="o", bufs=6))
        psA = ctx.enter_context(tc.tile_pool(name="psA", bufs=3, space="PSUM"))
        psV = ctx.enter_context(tc.tile_pool(name="psV", bufs=3, space="PSUM"))
        psT = ctx.enter_context(tc.tile_pool(name="psT", bufs=2, space="PSUM"))

        ident_f = const.tile([128, 128], f32)
        make_identity(nc, ident_f)
        ident_h = const.tile([128, 128], f16)
        nc.scalar.copy(ident_h, ident_f)
        # transposed-orientation causal mask: keep (partition=k_rel) <= (free=q_rel)
        triT = const.tile([128, 128], f32)
        nc.gpsimd.memset(triT, 0.0)
        nc.gpsimd.affine_select(
            out=triT, in_=triT, compare_op=mybir.AluOpType.is_ge, fill=NEG,
            base=0, pattern=[[1, 128]], channel_multiplier=-1)
        ones_col = const.tile([128, 1], f16)
        nc.vector.memset(ones_col, 1.0)
        ones_row = const.tile([1, 128], f16)
        nc.vector.memset(ones_row, 1.0)

        w_sb = wpool.tile([128, NCC, 6, HD], f16)
        for cc in range(NCC):   # split so the first matmul doesn't wait 3MB
            nc.sync.dma_start(w_sb[:, cc, :, :], wqkv_v[:, cc, :, :])
        wp_sb = wpool.tile([128, 2, C], f16)
        nc.sync.dma_start(wp_sb, wproj_v)

        def emit_qkv_chunk(b, tch, qkv_tiles):
            qt, kt, vt, v = qkv_tiles
            t0 = b * T + tch * 512
            xt_t = xtp.tile([128, NCC, 512], f16, tag="xt")
            for cc in range(NCC):
                nc.sync.dma_start(
                    xt_t[:, cc, :], xT[cc * 128:(cc + 1) * 128, t0:t0 + 512])
            for jj in range(6):  # q_h0, q_h1, k_h0, k_h1, v_h0, v_h1
                qk_ps = psA.tile([128, 512], f32, tag="psA")
                for cc in range(NCC):
                    nc.tensor.matmul(qk_ps, w_sb[:, cc, jj, :], xt_t[:, cc, :],
                                     start=(cc == 0), stop=(cc == NCC - 1))
                dst = (qt, qt, kt, kt, vt, vt)[jj]
                nc.scalar.copy(dst[:, jj % 2, tch * 512:(tch + 1) * 512], qk_ps)
            # transpose this chunk's V^T slice -> V [t, d]
            for hh in range(HPC):
                for tb in range(4):
                    tg = tch * 4 + tb
                    vp = psT.tile([128, 128], f16, tag="psT")
                    nc.tensor.transpose(
                        vp, vt[:, hh, tg * 128:(tg + 1) * 128], ident_h)
                    nc.vector.tensor_copy(v[:, tg, hh * HD:(hh + 1) * HD], vp)

        def emit_attn_mm(b, qg, h, qkv_tiles):
            """S matmuls + exp + PV accumulation for one (batch, q-group,
            head) unit. The softmax epilogue is deferred (see emit_epi*)."""
            qt, kt, vt, v = qkv_tiles
            pt_sb = ptp.tile([128, T // 128, 512], f16, tag="pt")
            # zero the stale upper-triangle region of the diagonal blocks so
            # the deferred denominator reduce can read whole blocks
            nc.vector.memset(pt_sb[:, 4 * qg:4 * qg + 4, :], 0.0)
            yt_ps = psV.tile([128, 512], f32, tag="psV")
            nkb = 4 * qg + 4
            for kb in range(nkb):
                kk = kb - 4 * qg
                qs = max(0, kk) * 128
                st = psA.tile([128, 512], f32, tag="psA")
                nc.tensor.matmul(
                    st[:, qs:512], kt[:, h, kb * 128:(kb + 1) * 128],
                    qt[:, h, qg * 512 + qs:(qg + 1) * 512],
                    start=True, stop=True)
                if kk >= 0:
                    nc.vector.tensor_add(
                        st[:, qs:qs + 128], st[:, qs:qs + 128], triT)
                nc.scalar.activation(
                    pt_sb[:, kb, qs:512], st[:, qs:512], Exp, scale=SCALE)
                nc.tensor.matmul(
                    yt_ps[:, qs:512], v[:, kb, h * HD:(h + 1) * HD],
                    pt_sb[:, kb, qs:512],
                    start=(kb == 0), stop=(kb == nkb - 1))
            return {"b": b, "qg": qg, "h": h, "pt": pt_sb, "yt_ps": yt_ps,
                    "nkb": nkb}

        def emit_epiA(u):
            """Denominator: DVE sum of P over k-blocks, one ones-matmul for
            the partition reduction, fast reciprocal. Emitted one unit after
            u's matmuls so the DVE work overlaps the next unit's PE work."""
            den_f = dnp.tile([128, 512], f32, tag="den")
            pv = u["pt"].rearrange("p a b -> p b a")[:, :, 0:u["nkb"]]
            nc.vector.tensor_reduce(den_f, pv, axis=AXX, op=Add)
            den16 = dnp.tile([128, 512], f16, tag="den16")
            nc.scalar.copy(den16, den_f)
            den_row = psA.tile([1, 512], f32, tag="psA")
            nc.tensor.matmul(den_row, ones_col, den16, start=True, stop=True)
            rec_sb = dnp.tile([1, 512], f32, tag="rec")
            nc.vector.reciprocal_approx_fast(rec_sb, den_row[0:1, :])
            rec16 = dnp.tile([1, 512], f16, tag="rec16")
            nc.scalar.copy(rec16, rec_sb)
            u["rec16"] = rec16

        def emit_epiB(u, yt):
            """Broadcast 1/den across partitions (PE) and normalize y^T.
            Emitted two units after u's matmuls: the reciprocal has had a
            full unit of slack, so the PE does not stall on the DVE chain."""
            r_ps = psA.tile([128, 512], f32, tag="psA")
            nc.tensor.matmul(r_ps, ones_row, u["rec16"], start=True, stop=True)
            r_sb = rp.tile([128, 512], f32, tag="rsb")
            nc.scalar.copy(r_sb, r_ps)
            nc.vector.tensor_mul(yt[:, u["h"], :], u["yt_ps"], r_sb)

        def emit_proj(b, qg, yt):
            for tt in range(4):
                for co in range(4):
                    o_ps = psA.tile([128, 512], f32, tag="psA")
                    for jh in range(HPC):
                        nc.tensor.matmul(
                            o_ps, yt[:, jh, tt * 128:(tt + 1) * 128],
                            wp_sb[:, jh, co * 512:(co + 1) * 512],
                            start=(jh == 0), stop=(jh == HPC - 1))
                    o_sb = op.tile([128, 512], f16, tag="osb")
                    # alternate PSUM evacuation between DVE and ACT so
                    # neither becomes the PSUM ring's bottleneck
                    if (tt * 4 + co) % 2 == 0:
                        nc.vector.tensor_copy(o_sb, o_ps)
                    else:
                        nc.scalar.copy(o_sb, o_ps)
                    r0 = b * T + qg * 512 + tt * 128
                    nc.sync.dma_start(
                        out[r0:r0 + 128, co * 512:(co + 1) * 512], o_sb)

        def alloc_qkv_tiles():
            qt = qkvp.tile([128, HPC, T], f16, tag="qt")
            kt = qkvp.tile([128, HPC, T], f16, tag="kt")
            vt = qkvp.tile([128, HPC, T], f16, tag="vt")
            v = qkvp.tile([128, T // 128, HPC * HD], f16, tag="v")
            return (qt, kt, vt, v)

        # Pipeline: QKV chunks of batch b+1 interleave into batch b's
        # attention stream; softmax epilogues trail their unit by 1 (epiA)
        # and 2 (epiB) units so the PE never waits on DVE/ACT results.
        tiles = alloc_qkv_tiles()
        for tch in range(NTCH):
            emit_qkv_chunk(0, tch, tiles)
        prevA = None   # unit awaiting epiA
        prevB = None   # unit awaiting epiB
        yts = {}       # (b, qg) -> yt tile
        for b in range(B):
            nxt = alloc_qkv_tiles() if b + 1 < B else None
            for qg in range(4):
                for h in range(HPC):
                    if h == 0 and nxt is not None:
                        emit_qkv_chunk(b + 1, qg, nxt)
                    u = emit_attn_mm(b, qg, h, tiles)
                    if h == 0:
                        yts[(b, qg)] = ytp.tile(
                            [128, HPC, 512], f16, tag="yt", name=f"yt{b}{qg}")
                    if prevA is not None:
                        emit_epiA(prevA)
                    if prevB is not None:
                        emit_epiB(prevB, yts[(prevB["b"], prevB["qg"])])
                        if prevB["h"] == 1:
                            emit_proj(prevB["b"], prevB["qg"],
                                      yts.pop((prevB["b"], prevB["qg"])))
                    prevB = prevA
                    prevA = u
            tiles = nxt
        # drain the epilogue pipeline
        emit_epiA(prevA)
        emit_epiB(prevB, yts[(prevB["b"], prevB["qg"])])
        if prevB["h"] == 1:
            emit_proj(prevB["b"], prevB["qg"],
                      yts.pop((prevB["b"], prevB["qg"])))
        emit_epiB(prevA, yts[(prevA["b"], prevA["qg"])])
        if prevA["h"] == 1:
            emit_proj(prevA["b"], prevA["qg"],
                      yts.pop((prevA["b"], prevA["qg"])))

    nc.compile()
    return nc


def _get_nc():
    if "nc" not in _CACHE:
        _CACHE["nc"] = _build_nc()
    return _CACHE["nc"]


def _make_in_maps(x2d, Wqkv, Wproj):
    xT = np.ascontiguousarray(x2d.T).astype(np.float16)  # [C, B*T]
    in_maps = []
    for c in range(N_CORES):
        h0 = c * HPC
        cols = []
        for part in range(3):  # q, k, v blocks of Wqkv columns
            for h in range(HPC):
                j0 = part * C + (h0 + h) * HD
                cols.append(Wqkv[:, j0:j0 + HD])
        wq = np.ascontiguousarray(np.concatenate(cols, axis=1)).astype(np.float16)
        wp = np.ascontiguousarray(
            Wproj[h0 * HD:(h0 + HPC) * HD, :]).astype(np.float16)
        in_maps.append({"xt": xT, "wqkv": wq, "wproj": wp})
    return in_maps


def run_shards(in_maps, trace=False):
    from concourse.bass_utils import run_bass_kernel_spmd
    nc = _get_nc()
    last_err = None
    for _attempt in range(3):
        try:
            return run_bass_kernel_spmd(
                nc, in_maps, core_ids=list(range(N_CORES)), trace=trace)
        except Exception as e:  # transient NRT device errors — retry
            last_err = e
            if "UNAVAILABLE" not in str(e) and "UNRECOVERABLE" not in str(e):
                raise
    raise last_err


def kernel(x, Wqkv, Wproj):
    x = np.asarray(x, dtype=np.float32)
    Wqkv = np.asarray(Wqkv, dtype=np.float32)
    Wproj = np.asarray(Wproj, dtype=np.float32)
    x2d = np.ascontiguousarray(x.reshape(B * T, C))

    in_maps = _make_in_maps(x2d, Wqkv, Wproj)
    res = run_shards(in_maps)

    acc = res.results[0]["out"].astype(np.float32)
    for c in range(1, N_CORES):
        acc += res.results[c]["out"].astype(np.float32)
    return acc.reshape(B, T, C)


# revision 6
# speedup vs baseline: 1.2650x; 1.2650x over previous
"""Causal self-attention (B=4, T=2048, C=2048, H=16) on 8 trn2 NeuronCores.

Sharding: tensor-parallel over heads — 2 heads per core. Every core gets the
full (pre-transposed) activation xT, its 2 heads' slice of Wqkv columns and
Wproj rows, computes a full [B*T, C] partial output (fp16), and the host sums
the 8 partials (the "all-reduce after output projection" done host-side).

Per-core dataflow (all matmuls fp16 on PE):
  xT tiles --DMA--> QKV proj -> Q^T,K^T [d,t] + V [t,d]
  S = Q^T.T @ K^T chunks (PSUM f32) -> +causal mask -> exp (ACT) -> P (fp16)
  y^T = sum_k V_k^T-block @ P^T-block (PSUM f32, accumulated over k-blocks)
  softmax denominator: DVE reduce of P over k-blocks -> one ones-matmul ->
  reciprocal_approx_fast -> PE row-broadcast -> DVE normalize.
  The den/rec/normalize epilogue is software-pipelined two attention units
  deep so the in-order PE queue never waits on the DVE/ACT chain (a PE stall
  also drops the PE to its half-speed p-state for ~3us).
  out_partial = y^T.T @ Wproj-rows (accumulate 2 head-chunks) -> fp16 -> DMA
"""
import numpy as np

B, T, C = 4, 2048, 2048
H, HD = 16, 128
N_CORES = 8
HPC = H // N_CORES          # heads per core = 2
SCALE = float(1.0 / np.sqrt(HD))
NEG = -1e9

_CACHE = {}


def _build_nc():
    import concourse.bass as bass
    from concourse import bacc
    import concourse.tile as tile
    import concourse.mybir as mybir
    from concourse.masks import make_identity
    from contextlib import ExitStack

    f32 = mybir.dt.float32
    f16 = mybir.dt.float16
    Exp = mybir.ActivationFunctionType.Exp
    AXX = mybir.AxisListType.X
    Add = mybir.AluOpType.add

    nc = bacc.Bacc("TRN2", target_bir_lowering=False, debug=False,
                   enable_asserts=True, num_devices=N_CORES)

    # Inputs (per-core shards prepared on host)
    xT = nc.dram_tensor("xt", [C, B * T], f16, kind="ExternalInput").ap()
    wqkv = nc.dram_tensor("wqkv", [C, 6 * HD], f16, kind="ExternalInput").ap()
    wproj = nc.dram_tensor("wproj", [HPC * HD, C], f16, kind="ExternalInput").ap()
    out = nc.dram_tensor("out", [B * T, C], f16, kind="ExternalOutput").ap()

    # DRAM views: c-chunked weights
    wqkv_v = wqkv.rearrange("(cc p) (jj d) -> p cc jj d", p=128, d=HD)  # [128,16,6,128]
    wproj_v = wproj.rearrange("(jh p) c -> p jh c", p=128)              # [128,2,2048]

    NCC = C // 128        # 16 contraction chunks
    NTCH = T // 512       # 4 t-chunks per batch

    with tile.TileContext(nc) as tc, ExitStack() as ctx:
        const = ctx.enter_context(tc.tile_pool(name="const", bufs=1))
        wpool = ctx.enter_context(tc.tile_pool(name="w", bufs=1))
        xtp = ctx.enter_context(tc.tile_pool(name="xt", bufs=2))
        qkvp = ctx.enter_context(tc.tile_pool(name="qkv", bufs=2))

        dnp = ctx.enter_context(tc.tile_pool(name="dn", bufs=2))
        rp = ctx.enter_context(tc.tile_pool(name="r", bufs=2))
        ptp = ctx.enter_context(tc.tile_pool(name="pt", bufs=2))
        ytp = ctx.enter_context(tc.tile_pool(name="yt", bufs=2))
        op = ctx.enter_context(tc.tile_pool(name="o", bufs=6))
        psA = ctx.enter_context(tc.tile_pool(name="psA", bufs=3, space="PSUM"))
        psV = ctx.enter_context(tc.tile_pool(name="psV", bufs=3, space="PSUM"))
        psT = ctx.enter_context(tc.tile_pool(name="psT", bufs=2, space="PSUM"))

        ident_f = const.tile([128, 128], f32)
        make_identity(nc, ident_f)
        ident_h = const.tile([128, 128], f16)
        nc.scalar.copy(ident_h, ident_f)
        # transposed-orientation causal mask: keep (partition=k_rel) <= (free=q_rel)
        triT = const.tile([128, 128], f32)
        nc.gpsimd.memset(triT, 0.0)
        nc.gpsimd.affine_select(
            out=triT, in_=triT, compare_op=mybir.AluOpType.is_ge, fill=NEG,
            base=0, pattern=[[1, 128]], channel_multiplier=-1)
        ones_col = const.tile([128, 1], f16)
        nc.vector.memset(ones_col, 1.0)
        ones_row = const.tile([1, 128], f16)
        nc.vector.memset(ones_row, 1.0)

        w_sb = wpool.tile([128, NCC, 6, HD], f16)
        for cc in range(NCC):   # split so the first matmul doesn't wait 3MB
            nc.sync.dma_start(w_sb[:, cc, :, :], wqkv_v[:, cc, :, :])
        wp_sb = wpool.tile([128, 2, C], f16)
        nc.sync.dma_start(wp_sb, wproj_v)

        def emit_qkv_chunk(b, tch, qkv_tiles):
            qt, kt, vt, v = qkv_tiles
            t0 = b * T + tch * 512
            xt_t = xtp.tile([128, NCC, 512], f16, tag="xt")
            for cc in range(NCC):
                nc.sync.dma_start(
                    xt_t[:, cc, :], xT[cc * 128:(cc + 1) * 128, t0:t0 + 512])
            for jj in range(6):  # q_h0, q_h1, k_h0, k_h1, v_h0, v_h1
                qk_ps = psA.tile([128, 512], f32, tag="psA")
                for cc in range(NCC):
                    nc.tensor.matmul(qk_ps, w_sb[:, cc, jj, :], xt_t[:, cc, :],
                                     start=(cc == 0), stop=(cc == NCC - 1))
                dst = (qt, qt, kt, kt, vt, vt)[jj]
                nc.scalar.copy(dst[:, jj % 2, tch * 512:(tch + 1) * 512], qk_ps)
            # transpose this chunk's V^T slice -> V [t, d]
            for hh in range(HPC):
                for tb in range(4):
                    tg = tch * 4 + tb
                    vp = psT.tile([128, 128], f16, tag="psT")
                    nc.tensor.transpose(
                        vp, vt[:, hh, tg * 128:(tg + 1) * 128], ident_h)
                    nc.vector.tensor_copy(v[:, tg, hh * HD:(hh + 1) * HD], vp)

        def emit_attn_mm(b, qg, h, qkv_tiles):
            """S matmuls + exp + PV accumulation for one (batch, q-group,
            head) unit. The softmax epilogue is deferred (see emit_epi*)."""
            qt, kt, vt, v = qkv_tiles
            pt_sb = ptp.tile([128, T // 128, 512], f16, tag="pt")
            den_f = dnp.tile([128, 512], f32, tag="den")
            yt_ps = psV.tile([128, 512], f32, tag="psV")
            nkb = 4 * qg + 4
            for kb in range(nkb):
                kk = kb - 4 * qg
                qs = max(0, kk) * 128
                st = psA.tile([128, 512], f32, tag="psA")
                nc.tensor.matmul(
                    st[:, qs:512], kt[:, h, kb * 128:(kb + 1) * 128],
                    qt[:, h, qg * 512 + qs:(qg + 1) * 512],
                    start=True, stop=True)
                if kk >= 0:
                    nc.vector.tensor_add(
                        st[:, qs:qs + 128], st[:, qs:qs + 128], triT)
                nc.scalar.activation(
                    pt_sb[:, kb, qs:512], st[:, qs:512], Exp, scale=SCALE)
                # running denominator partial sum (per k-partition) on DVE,
                # chasing the exp stream; contiguous reads, no stale regions
                if kb == 0:
                    nc.vector.tensor_copy(den_f, pt_sb[:, 0, :])
                else:
                    nc.vector.tensor_add(
                        den_f[:, qs:512], den_f[:, qs:512],
                        pt_sb[:, kb, qs:512])
                nc.tensor.matmul(
                    yt_ps[:, qs:512], v[:, kb, h * HD:(h + 1) * HD],
                    pt_sb[:, kb, qs:512],
                    start=(kb == 0), stop=(kb == nkb - 1))
            return {"b": b, "qg": qg, "h": h, "den_f": den_f, "yt_ps": yt_ps,
                    "nkb": nkb}

        def emit_epiA(u):
            """Denominator: one ones-matmul for the partition reduction of
            the DVE-accumulated partial sums, then fast reciprocal. Emitted
            one unit after u's matmuls so it overlaps the next unit's PE
            work."""
            den16 = dnp.tile([128, 512], f16, tag="den16")
            nc.scalar.copy(den16, u["den_f"])
            den_row = psA.tile([1, 512], f32, tag="psA")
            nc.tensor.matmul(den_row, ones_col, den16, start=True, stop=True)
            rec_sb = dnp.tile([1, 512], f32, tag="rec")
            nc.vector.reciprocal_approx_fast(rec_sb, den_row[0:1, :])
            rec16 = dnp.tile([1, 512], f16, tag="rec16")
            nc.scalar.copy(rec16, rec_sb)
            u["rec16"] = rec16

        def emit_epiB(u, yt):
            """Broadcast 1/den across partitions (PE) and normalize y^T.
            Emitted two units after u's matmuls: the reciprocal has had a
            full unit of slack, so the PE does not stall on the DVE chain."""
            r_ps = psA.tile([128, 512], f32, tag="psA")
            nc.tensor.matmul(r_ps, ones_row, u["rec16"], start=True, stop=True)
            r_sb = rp.tile([128, 512], f32, tag="rsb")
            nc.scalar.copy(r_sb, r_ps)
            nc.vector.tensor_mul(yt[:, u["h"], :], u["yt_ps"], r_sb)

        def emit_proj(b, qg, yt):
            for tt in range(4):
                for co in range(4):
                    o_ps = psA.tile([128, 512], f32, tag="psA")
                    for jh in range(HPC):
                        nc.tensor.matmul(
                            o_ps, yt[:, jh, tt * 128:(tt + 1) * 128],
                            wp_sb[:, jh, co * 512:(co + 1) * 512],
                            start=(jh == 0), stop=(jh == HPC - 1))
                    o_sb = op.tile([128, 512], f16, tag="osb")
                    # alternate PSUM evacuation between DVE and ACT so
                    # neither becomes the PSUM ring's bottleneck
                    if (tt * 4 + co) % 2 == 0:
                        nc.vector.tensor_copy(o_sb, o_ps)
                    else:
                        nc.scalar.copy(o_sb, o_ps)
                    r0 = b * T + qg * 512 + tt * 128
                    nc.sync.dma_start(
                        out[r0:r0 + 128, co * 512:(co + 1) * 512], o_sb)

        def alloc_qkv_tiles():
            qt = qkvp.tile([128, HPC, T], f16, tag="qt")
            kt = qkvp.tile([128, HPC, T], f16, tag="kt")
            vt = qkvp.tile([128, HPC, T], f16, tag="vt")
            v = qkvp.tile([128, T // 128, HPC * HD], f16, tag="v")
            return (qt, kt, vt, v)

        # Pipeline: QKV chunks of batch b+1 interleave into batch b's
        # attention stream; softmax epilogues trail their unit by 1 (epiA)
        # and 2 (epiB) units so the PE never waits on DVE/ACT results.
        tiles = alloc_qkv_tiles()
        for tch in range(NTCH):
            emit_qkv_chunk(0, tch, tiles)
        prevA = None   # unit awaiting epiA
        prevB = None   # unit awaiting epiB
        yts = {}       # (b, qg) -> yt tile
        for b in range(B):
            nxt = alloc_qkv_tiles() if b + 1 < B else None
            for qg in range(4):
                for h in range(HPC):
                    if h == 0 and nxt is not None:
                        emit_qkv_chunk(b + 1, qg, nxt)
                    u = emit_attn_mm(b, qg, h, tiles)
                    if h == 0:
                        yts[(b, qg)] = ytp.tile(
                            [128, HPC, 512], f16, tag="yt", name=f"yt{b}{qg}")
                    if prevA is not None:
                        emit_epiA(prevA)
                    if prevB is not None:
                        emit_epiB(prevB, yts[(prevB["b"], prevB["qg"])])
                        if prevB["h"] == 1:
                            emit_proj(prevB["b"], prevB["qg"],
                                      yts.pop((prevB["b"], prevB["qg"])))
                    prevB = prevA
                    prevA = u
            tiles = nxt
        # drain the epilogue pipeline
        emit_epiA(prevA)
        emit_epiB(prevB, yts[(prevB["b"], prevB["qg"])])
        if prevB["h"] == 1:
            emit_proj(prevB["b"], prevB["qg"],
                      yts.pop((prevB["b"], prevB["qg"])))
        emit_epiB(prevA, yts[(prevA["b"], prevA["qg"])])
        if prevA["h"] == 1:
            emit_proj(prevA["b"], prevA["qg"],
                      yts.pop((prevA["b"], prevA["qg"])))

    nc.compile()
    return nc


def _get_nc():
    if "nc" not in _CACHE:
        _CACHE["nc"] = _build_nc()
    return _CACHE["nc"]


def _make_in_maps(x2d, Wqkv, Wproj):
    xT = np.ascontiguousarray(x2d.T).astype(np.float16)  # [C, B*T]
    in_maps = []
    for c in range(N_CORES):
        h0 = c * HPC
        cols = []
        for part in range(3):  # q, k, v blocks of Wqkv columns
            for h in range(HPC):
                j0 = part * C + (h0 + h) * HD
                cols.append(Wqkv[:, j0:j0 + HD])
        wq = np.ascontiguousarray(np.concatenate(cols, axis=1)).astype(np.float16)
        wp = np.ascontiguousarray(
            Wproj[h0 * HD:(h0 + HPC) * HD, :]).astype(np.float16)
        in_maps.append({"xt": xT, "wqkv": wq, "wproj": wp})
    return in_maps


def run_shards(in_maps, trace=False):
    from concourse.bass_utils import run_bass_kernel_spmd
    nc = _get_nc()
    last_err = None
    for _attempt in range(3):
        try:
            return run_bass_kernel_spmd(
                nc, in_maps, core_ids=list(range(N_CORES)), trace=trace)
        except Exception as e:  # transient NRT device errors — retry
            last_err = e
            if "UNAVAILABLE" not in str(e) and "UNRECOVERABLE" not in str(e):
                raise
    raise last_err


def kernel(x, Wqkv, Wproj):
    x = np.asarray(x, dtype=np.float32)
    Wqkv = np.asarray(Wqkv, dtype=np.float32)
    Wproj = np.asarray(Wproj, dtype=np.float32)
    x2d = np.ascontiguousarray(x.reshape(B * T, C))

    in_maps = _make_in_maps(x2d, Wqkv, Wproj)
    res = run_shards(in_maps)

    acc = res.results[0]["out"].astype(np.float32)
    for c in range(1, N_CORES):
        acc += res.results[c]["out"].astype(np.float32)
    return acc.reshape(B, T, C)


# revision 11
# speedup vs baseline: 1.2956x; 1.0242x over previous
"""Causal self-attention (B=4, T=2048, C=2048, H=16) on 8 trn2 NeuronCores.

Sharding: tensor-parallel over heads — 2 heads per core. Every core gets the
full (pre-transposed) activation xT, its 2 heads' slice of Wqkv columns and
Wproj rows, computes a full [B*T, C] partial output (fp16), and the host sums
the 8 partials (the "all-reduce after output projection" done host-side).

Per-core dataflow (all matmuls fp16 on PE):
  xT tiles --DMA--> QKV proj -> Q^T,K^T [d,t] + V [t,d]
  S = Q^T.T @ K^T chunks (PSUM f32) -> +causal mask -> exp (ACT) -> P (fp16)
  y^T = sum_k V_k^T-block @ P^T-block (PSUM f32, accumulated over k-blocks)
  softmax denominator: DVE reduce of P over k-blocks -> one ones-matmul ->
  reciprocal_approx_fast -> PE row-broadcast -> DVE normalize.
  The den/rec/normalize epilogue is software-pipelined two attention units
  deep so the in-order PE queue never waits on the DVE/ACT chain (a PE stall
  also drops the PE to its half-speed p-state for ~3us).
  out_partial = y^T.T @ Wproj-rows (accumulate 2 head-chunks) -> fp16 -> DMA
"""
import numpy as np

B, T, C = 4, 2048, 2048
H, HD = 16, 128
N_CORES = 8
HPC = H // N_CORES          # heads per core = 2
SCALE = float(1.0 / np.sqrt(HD))
NEG = -1e9

_CACHE = {}


def _build_nc():
    import concourse.bass as bass
    from concourse import bacc
    import concourse.tile as tile
    import concourse.mybir as mybir
    from concourse.masks import make_identity
    from contextlib import ExitStack

    f32 = mybir.dt.float32
    f16 = mybir.dt.float16
    Exp = mybir.ActivationFunctionType.Exp
    AXX = mybir.AxisListType.X
    Add = mybir.AluOpType.add

    nc = bacc.Bacc("TRN2", target_bir_lowering=False, debug=False,
                   enable_asserts=True, num_devices=N_CORES)

    # Inputs (per-core shards prepared on host)
    xT = nc.dram_tensor("xt", [C, B * T], f16, kind="ExternalInput").ap()
    wqkv = nc.dram_tensor("wqkv", [C, 6 * HD], f16, kind="ExternalInput").ap()
    wproj = nc.dram_tensor("wproj", [HPC * HD, C], f16, kind="ExternalInput").ap()
    out = nc.dram_tensor("out", [B * T, C], f16, kind="ExternalOutput").ap()

    # DRAM views: c-chunked weights
    wqkv_v = wqkv.rearrange("(cc p) (jj d) -> p cc jj d", p=128, d=HD)  # [128,16,6,128]
    wproj_v = wproj.rearrange("(jh p) c -> p jh c", p=128)              # [128,2,2048]

    NCC = C // 128        # 16 contraction chunks
    NTCH = T // 512       # 4 t-chunks per batch

    with tile.TileContext(nc) as tc, ExitStack() as ctx:
        const = ctx.enter_context(tc.tile_pool(name="const", bufs=1))
        wpool = ctx.enter_context(tc.tile_pool(name="w", bufs=1))
        xtp = ctx.enter_context(tc.tile_pool(name="xt", bufs=2))
        qkvp = ctx.enter_context(tc.tile_pool(name="qkv", bufs=2))

        dnp = ctx.enter_context(tc.tile_pool(name="dn", bufs=2))
        rp = ctx.enter_context(tc.tile_pool(name="r", bufs=2))
        ptp = ctx.enter_context(tc.tile_pool(name="pt", bufs=2))
        ytp = ctx.enter_context(tc.tile_pool(name="yt", bufs=2))
        op = ctx.enter_context(tc.tile_pool(name="o", bufs=6))
        psA = ctx.enter_context(tc.tile_pool(name="psA", bufs=3, space="PSUM"))
        psV = ctx.enter_context(tc.tile_pool(name="psV", bufs=3, space="PSUM"))
        psT = ctx.enter_context(tc.tile_pool(name="psT", bufs=2, space="PSUM"))

        ident_f = const.tile([128, 128], f32)
        make_identity(nc, ident_f)
        ident_h = const.tile([128, 128], f16)
        nc.scalar.copy(ident_h, ident_f)
        # causal mask as a PE accumulation: st[k,q] += -60000 * (k > q).
        # maskL = -60000*I (stationary), maskU[c,q] = 1 where c > q (moving);
        # keeps the mask on the Tensor engine - no DVE hop in the S->exp chain
        maskL = const.tile([128, 128], f16)
        nc.scalar.mul(maskL, ident_f, -60000.0)
        mU32 = const.tile([128, 128], f32)
        nc.gpsimd.memset(mU32, 1.0)
        nc.gpsimd.affine_select(
            out=mU32, in_=mU32, compare_op=mybir.AluOpType.is_ge, fill=0.0,
            base=-1, pattern=[[-1, 128]], channel_multiplier=1)
        maskU = const.tile([128, 128], f16)
        nc.scalar.copy(maskU, mU32)
        ones_col = const.tile([128, 1], f16)
        nc.vector.memset(ones_col, 1.0)
        ones_row = const.tile([1, 128], f16)
        nc.vector.memset(ones_row, 1.0)

        w_sb = wpool.tile([128, NCC, 6, HD], f16)
        wp_sb = wpool.tile([128, 2, C], f16)

        def qkv_dma(b, tch):
            t0 = b * T + tch * 512
            xt_t = xtp.tile([128, NCC, 512], f16, tag="xt",
                            name=f"xt{b}{tch}")
            for cc in range(NCC):
                nc.sync.dma_start(
                    xt_t[:, cc, :], xT[cc * 128:(cc + 1) * 128, t0:t0 + 512])
            return xt_t

        def qkv_mm(b, tch, xt_t, qkv_tiles):
            qt, kt, vt, v = qkv_tiles
            for jj in range(6):  # q_h0, q_h1, k_h0, k_h1, v_h0, v_h1
                qk_ps = psA.tile([128, 512], f32, tag="psA")
                for cc in range(NCC):
                    nc.tensor.matmul(qk_ps, w_sb[:, cc, jj, :], xt_t[:, cc, :],
                                     start=(cc == 0), stop=(cc == NCC - 1))
                dst = (qt, qt, kt, kt, vt, vt)[jj]
                nc.scalar.copy(dst[:, jj % 2, tch * 512:(tch + 1) * 512], qk_ps)
            # transpose this chunk's V^T slice -> V [t, d]
            for hh in range(HPC):
                for tb in range(4):
                    tg = tch * 4 + tb
                    vp = psT.tile([128, 128], f16, tag="psT")
                    nc.tensor.transpose(
                        vp, vt[:, hh, tg * 128:(tg + 1) * 128], ident_h)
                    nc.vector.tensor_copy(v[:, tg, hh * HD:(hh + 1) * HD], vp)

        def emit_attn_mm(b, qg, h, qkv_tiles):
            """S matmuls + exp + PV accumulation for one (batch, q-group,
            head) unit. The softmax epilogue is deferred (see emit_epi*)."""
            qt, kt, vt, v = qkv_tiles
            pt_sb = ptp.tile([128, T // 128, 512], f16, tag="pt")
            den_f = dnp.tile([128, 512], f32, tag="den")
            yt_ps = psV.tile([128, 512], f32, tag="psV")
            nkb = 4 * qg + 4
            for kb in range(nkb):
                kk = kb - 4 * qg
                qs = max(0, kk) * 128
                st = psA.tile([128, 512], f32, tag="psA")
                nc.tensor.matmul(
                    st[:, qs:512], kt[:, h, kb * 128:(kb + 1) * 128],
                    qt[:, h, qg * 512 + qs:(qg + 1) * 512],
                    start=True, stop=(kk < 0))
                if kk >= 0:
                    nc.tensor.matmul(
                        st[:, qs:qs + 128], maskL, maskU,
                        start=False, stop=True)
                nc.scalar.activation(
                    pt_sb[:, kb, qs:512], st[:, qs:512], Exp, scale=SCALE)
                # running denominator partial sum (per k-partition) on DVE,
                # chasing the exp stream; contiguous reads, no stale regions
                if kb == 0:
                    nc.vector.tensor_copy(den_f, pt_sb[:, 0, :])
                else:
                    nc.vector.tensor_add(
                        den_f[:, qs:512], den_f[:, qs:512],
                        pt_sb[:, kb, qs:512])
                nc.tensor.matmul(
                    yt_ps[:, qs:512], v[:, kb, h * HD:(h + 1) * HD],
                    pt_sb[:, kb, qs:512],
                    start=(kb == 0), stop=(kb == nkb - 1))
            return {"b": b, "qg": qg, "h": h, "den_f": den_f, "yt_ps": yt_ps,
                    "nkb": nkb}

        def emit_epiA(u):
            """Denominator: one ones-matmul for the partition reduction of
            the DVE-accumulated partial sums, then fast reciprocal. Emitted
            one unit after u's matmuls so it overlaps the next unit's PE
            work."""
            den16 = dnp.tile([128, 512], f16, tag="den16")
            nc.vector.tensor_copy(den16, u["den_f"])
            den_row = psA.tile([1, 512], f32, tag="psA")
            nc.tensor.matmul(den_row, ones_col, den16, start=True, stop=True)
            rec_sb = dnp.tile([1, 512], f32, tag="rec")
            nc.vector.reciprocal_approx_fast(rec_sb, den_row[0:1, :])
            rec16 = dnp.tile([1, 512], f16, tag="rec16")
            nc.scalar.copy(rec16, rec_sb)
            u["rec16"] = rec16

        def emit_epiB(u, yt):
            """Broadcast 1/den across partitions (PE) and normalize y^T.
            Emitted two units after u's matmuls: the reciprocal has had a
            full unit of slack, so the PE does not stall on the DVE chain."""
            r_ps = psA.tile([128, 512], f32, tag="psA")
            nc.tensor.matmul(r_ps, ones_row, u["rec16"], start=True, stop=True)
            r_sb = rp.tile([128, 512], f32, tag="rsb")
            nc.scalar.copy(r_sb, r_ps)
            nc.vector.tensor_mul(yt[:, u["h"], :], u["yt_ps"], r_sb)

        def emit_proj(b, qg, yt):
            for tt in range(4):
                for co in range(4):
                    o_ps = psA.tile([128, 512], f32, tag="psA")
                    for jh in range(HPC):
                        nc.tensor.matmul(
                            o_ps, yt[:, jh, tt * 128:(tt + 1) * 128],
                            wp_sb[:, jh, co * 512:(co + 1) * 512],
                            start=(jh == 0), stop=(jh == HPC - 1))
                    o_sb = op.tile([128, 512], f16, tag="osb")
                    # alternate PSUM evacuation between DVE and ACT so
                    # neither becomes the PSUM ring's bottleneck
                    if (tt * 4 + co) % 2 == 0:
                        nc.vector.tensor_copy(o_sb, o_ps)
                    else:
                        nc.scalar.copy(o_sb, o_ps)
                    r0 = b * T + qg * 512 + tt * 128
                    nc.sync.dma_start(
                        out[r0:r0 + 128, co * 512:(co + 1) * 512], o_sb)

        def alloc_qkv_tiles():
            qt = qkvp.tile([128, HPC, T], f16, tag="qt")
            kt = qkvp.tile([128, HPC, T], f16, tag="kt")
            vt = qkvp.tile([128, HPC, T], f16, tag="vt")
            v = qkvp.tile([128, T // 128, HPC * HD], f16, tag="v")
            return (qt, kt, vt, v)

        # Pipeline: QKV chunks of batch b+1 interleave into batch b's
        # attention stream, with each chunk's xt DMA issued one chunk ahead
        # so the PE never waits on an in-flight transfer; softmax epilogues
        # trail their unit by 1 (epiA) and 2 (epiB) units so the PE never
        # waits on DVE/ACT results.
        chunk_after = {}
        _seq = [(b, t) for b in range(B) for t in range(NTCH)]
        for _i, _c in enumerate(_seq[:-1]):
            chunk_after[_c] = _seq[_i + 1]

        tiles = alloc_qkv_tiles()
        xt_pend = {}
        # startup: interleave weight-chunk and first-xt-chunk DMA issue so
        # the first matmul waits on two small transfers, not all of them
        xt00 = xtp.tile([128, NCC, 512], f16, tag="xt", name="xt00")
        xt_pend[(0, 0)] = xt00
        for cc in range(NCC):
            nc.sync.dma_start(w_sb[:, cc, :, :], wqkv_v[:, cc, :, :])
            nc.sync.dma_start(
                xt00[:, cc, :], xT[cc * 128:(cc + 1) * 128, 0:512])

        def run_chunk(bt):
            if bt in chunk_after:
                nb = chunk_after[bt]
                xt_pend[nb] = qkv_dma(*nb)
            qkv_mm(bt[0], bt[1], xt_pend.pop(bt),
                   tiles if bt[0] == cur_b else nxt)

        cur_b = 0
        nxt = tiles
        run_chunk((0, 0))
        nc.sync.dma_start(wp_sb, wproj_v)
        for tch in range(1, NTCH):
            run_chunk((0, tch))
        prevA = None   # unit awaiting epiA
        prevB = None   # unit awaiting epiB
        yts = {}       # (b, qg) -> yt tile
        for b in range(B):
            cur_b = b
            nxt = alloc_qkv_tiles() if b + 1 < B else None
            for qg in range(4):
                for h in range(HPC):
                    if h == 0 and nxt is not None:
                        run_chunk((b + 1, qg))
                    u = emit_attn_mm(b, qg, h, tiles)
                    if h == 0:
                        yts[(b, qg)] = ytp.tile(
                            [128, HPC, 512], f16, tag="yt", name=f"yt{b}{qg}")
                    if prevA is not None:
                        emit_epiA(prevA)
                    if prevB is not None:
                        emit_epiB(prevB, yts[(prevB["b"], prevB["qg"])])
                        if prevB["h"] == 1:
                            emit_proj(prevB["b"], prevB["qg"],
                                      yts.pop((prevB["b"], prevB["qg"])))
                    prevB = prevA
                    prevA = u
            tiles = nxt
        # drain the epilogue pipeline
        emit_epiA(prevA)
        emit_epiB(prevB, yts[(prevB["b"], prevB["qg"])])
        if prevB["h"] == 1:
            emit_proj(prevB["b"], prevB["qg"],
                      yts.pop((prevB["b"], prevB["qg"])))
        emit_epiB(prevA, yts[(prevA["b"], prevA["qg"])])
        if prevA["h"] == 1:
            emit_proj(prevA["b"], prevA["qg"],
                      yts.pop((prevA["b"], prevA["qg"])))

    nc.compile()
    return nc


def _get_nc():
    if "nc" not in _CACHE:
        _CACHE["nc"] = _build_nc()
    return _CACHE["nc"]


def _make_in_maps(x2d, Wqkv, Wproj):
    xT = np.ascontiguousarray(x2d.T).astype(np.float16)  # [C, B*T]
    in_maps = []
    for c in range(N_CORES):
        h0 = c * HPC
        cols = []
        for part in range(3):  # q, k, v blocks of Wqkv columns
            for h in range(HPC):
                j0 = part * C + (h0 + h) * HD
                cols.append(Wqkv[:, j0:j0 + HD])
        wq = np.ascontiguousarray(np.concatenate(cols, axis=1)).astype(np.float16)
        wp = np.ascontiguousarray(
            Wproj[h0 * HD:(h0 + HPC) * HD, :]).astype(np.float16)
        in_maps.append({"xt": xT, "wqkv": wq, "wproj": wp})
    return in_maps


def run_shards(in_maps, trace=False):
    from concourse.bass_utils import run_bass_kernel_spmd
    nc = _get_nc()
    last_err = None
    for _attempt in range(3):
        try:
            return run_bass_kernel_spmd(
                nc, in_maps, core_ids=list(range(N_CORES)), trace=trace)
        except Exception as e:  # transient NRT device errors — retry
            last_err = e
            if "UNAVAILABLE" not in str(e) and "UNRECOVERABLE" not in str(e):
                raise
    raise last_err


def kernel(x, Wqkv, Wproj):
    x = np.asarray(x, dtype=np.float32)
    Wqkv = np.asarray(Wqkv, dtype=np.float32)
    Wproj = np.asarray(Wproj, dtype=np.float32)
    x2d = np.ascontiguousarray(x.reshape(B * T, C))

    in_maps = _make_in_maps(x2d, Wqkv, Wproj)
    res = run_shards(in_maps)

    acc = res.results[0]["out"].astype(np.float32)
    for c in range(1, N_CORES):
        acc += res.results[c]["out"].astype(np.float32)
    return acc.reshape(B, T, C)


# revision 13
# speedup vs baseline: 1.3217x; 1.0201x over previous
"""Causal self-attention (B=4, T=2048, C=2048, H=16) on 8 trn2 NeuronCores.

Sharding: tensor-parallel over heads — 2 heads per core. Every core gets the
full (pre-transposed) activation xT, its 2 heads' slice of Wqkv columns and
Wproj rows, computes a full [B*T, C] partial output (fp16), and the host sums
the 8 partials (the "all-reduce after output projection" done host-side).

Per-core dataflow (all matmuls fp16 on PE):
  xT tiles --DMA--> QKV proj -> Q^T,K^T [d,t] + V [t,d]
  S = Q^T.T @ K^T chunks (PSUM f32) -> +causal mask -> exp (ACT) -> P (fp16)
  y^T = sum_k V_k^T-block @ P^T-block (PSUM f32, accumulated over k-blocks)
  softmax denominator: DVE reduce of P over k-blocks -> one ones-matmul ->
  reciprocal_approx_fast -> PE row-broadcast -> DVE normalize.
  The den/rec/normalize epilogue is software-pipelined two attention units
  deep so the in-order PE queue never waits on the DVE/ACT chain (a PE stall
  also drops the PE to its half-speed p-state for ~3us).
  out_partial = y^T.T @ Wproj-rows (accumulate 2 head-chunks) -> fp16 -> DMA
"""
import numpy as np

B, T, C = 4, 2048, 2048
H, HD = 16, 128
N_CORES = 8
HPC = H // N_CORES          # heads per core = 2
SCALE = float(1.0 / np.sqrt(HD))
NEG = -1e9

_CACHE = {}


def _build_nc():
    import concourse.bass as bass
    from concourse import bacc
    import concourse.tile as tile
    import concourse.mybir as mybir
    from concourse.masks import make_identity
    from contextlib import ExitStack

    f32 = mybir.dt.float32
    f16 = mybir.dt.float16
    Exp = mybir.ActivationFunctionType.Exp
    AXX = mybir.AxisListType.X
    Add = mybir.AluOpType.add

    nc = bacc.Bacc("TRN2", target_bir_lowering=False, debug=False,
                   enable_asserts=True, num_devices=N_CORES)

    # Inputs (per-core shards prepared on host)
    xT = nc.dram_tensor("xt", [C, B * T], f16, kind="ExternalInput").ap()
    wqkv = nc.dram_tensor("wqkv", [C, 6 * HD], f16, kind="ExternalInput").ap()
    wproj = nc.dram_tensor("wproj", [HPC * HD, C], f16, kind="ExternalInput").ap()
    out = nc.dram_tensor("out", [B * T, C], f16, kind="ExternalOutput").ap()

    # DRAM views: c-chunked weights
    wqkv_v = wqkv.rearrange("(cc p) (jj d) -> p cc jj d", p=128, d=HD)  # [128,16,6,128]
    wproj_v = wproj.rearrange("(jh p) c -> p jh c", p=128)              # [128,2,2048]

    NCC = C // 128        # 16 contraction chunks
    NTCH = T // 512       # 4 t-chunks per batch

    with tile.TileContext(nc) as tc, ExitStack() as ctx:
        const = ctx.enter_context(tc.tile_pool(name="const", bufs=1))
        wpool = ctx.enter_context(tc.tile_pool(name="w", bufs=1))
        xtp = ctx.enter_context(tc.tile_pool(name="xt", bufs=2))
        qkvp = ctx.enter_context(tc.tile_pool(name="qkv", bufs=2))

        dnp = ctx.enter_context(tc.tile_pool(name="dn", bufs=2))
        rp = ctx.enter_context(tc.tile_pool(name="r", bufs=2))
        ptp = ctx.enter_context(tc.tile_pool(name="pt", bufs=2))
        ytp = ctx.enter_context(tc.tile_pool(name="yt", bufs=2))
        op = ctx.enter_context(tc.tile_pool(name="o", bufs=6))
        psA = ctx.enter_context(tc.tile_pool(name="psA", bufs=4, space="PSUM"))
        psV = ctx.enter_context(tc.tile_pool(name="psV", bufs=3, space="PSUM"))
        psT = ctx.enter_context(tc.tile_pool(name="psT", bufs=1, space="PSUM"))

        ident_f = const.tile([128, 128], f32)
        make_identity(nc, ident_f)
        ident_h = const.tile([128, 128], f16)
        nc.scalar.copy(ident_h, ident_f)
        # causal mask as a PE accumulation: st[k,q] += -60000 * (k > q).
        # maskL = -60000*I (stationary), maskU[c,q] = 1 where c > q (moving);
        # keeps the mask on the Tensor engine - no DVE hop in the S->exp chain
        maskL = const.tile([128, 128], f16)
        nc.scalar.mul(maskL, ident_f, -60000.0)
        mU32 = const.tile([128, 128], f32)
        nc.gpsimd.memset(mU32, 1.0)
        nc.gpsimd.affine_select(
            out=mU32, in_=mU32, compare_op=mybir.AluOpType.is_ge, fill=0.0,
            base=-1, pattern=[[-1, 128]], channel_multiplier=1)
        maskU = const.tile([128, 128], f16)
        nc.scalar.copy(maskU, mU32)
        ones_col = const.tile([128, 1], f16)
        nc.vector.memset(ones_col, 1.0)
        ones_row = const.tile([1, 128], f16)
        nc.vector.memset(ones_row, 1.0)

        w_sb = wpool.tile([128, NCC, 6, HD], f16)
        wp_sb = wpool.tile([128, 2, C], f16)

        def qkv_dma(b, tch):
            t0 = b * T + tch * 512
            xt_t = xtp.tile([128, NCC, 512], f16, tag="xt",
                            name=f"xt{b}{tch}")
            for cc in range(NCC):
                nc.sync.dma_start(
                    xt_t[:, cc, :], xT[cc * 128:(cc + 1) * 128, t0:t0 + 512])
            return xt_t

        def qkv_mm(b, tch, xt_t, qkv_tiles):
            qt, kt, vt, v = qkv_tiles
            for jj in range(6):  # q_h0, q_h1, k_h0, k_h1, v_h0, v_h1
                qk_ps = psA.tile([128, 512], f32, tag="psA")
                for cc in range(NCC):
                    nc.tensor.matmul(qk_ps, w_sb[:, cc, jj, :], xt_t[:, cc, :],
                                     start=(cc == 0), stop=(cc == NCC - 1))
                dst = (qt, qt, kt, kt, vt, vt)[jj]
                nc.scalar.copy(dst[:, jj % 2, tch * 512:(tch + 1) * 512], qk_ps)
            # transpose this chunk's V^T slice -> V [t, d]
            for hh in range(HPC):
                for tb in range(4):
                    tg = tch * 4 + tb
                    vp = psT.tile([128, 128], f16, tag="psT")
                    nc.tensor.transpose(
                        vp, vt[:, hh, tg * 128:(tg + 1) * 128], ident_h)
                    nc.vector.tensor_copy(v[:, tg, hh * HD:(hh + 1) * HD], vp)

        def emit_attn_mm(b, qg, h, qkv_tiles):
            """S matmuls + exp + PV accumulation for one (batch, q-group,
            head) unit. The softmax epilogue is deferred (see emit_epi*)."""
            qt, kt, vt, v = qkv_tiles
            pt_sb = ptp.tile([128, T // 128, 512], f16, tag="pt")
            den_f = dnp.tile([128, 512], f32, tag="den")
            yt_ps = psV.tile([128, 512], f32, tag="psV")
            nkb = 4 * qg + 4
            DEPTH = 2   # S-blocks emitted ahead of their exp/PV consumers
            pend = []

            def flush_one():
                kb, qs, st = pend.pop(0)
                nc.scalar.activation(
                    pt_sb[:, kb, qs:512], st[:, qs:512], Exp, scale=SCALE)
                # running denominator partial sum (per k-partition) on DVE,
                # chasing the exp stream
                if kb == 0:
                    nc.vector.tensor_copy(den_f, pt_sb[:, 0, :])
                else:
                    nc.vector.tensor_add(
                        den_f[:, qs:512], den_f[:, qs:512],
                        pt_sb[:, kb, qs:512])
                nc.tensor.matmul(
                    yt_ps[:, qs:512], v[:, kb, h * HD:(h + 1) * HD],
                    pt_sb[:, kb, qs:512],
                    start=(kb == 0), stop=(kb == nkb - 1))

            for kb in range(nkb):
                kk = kb - 4 * qg
                qs = max(0, kk) * 128
                st = psA.tile([128, 512], f32, tag="psA")
                nc.tensor.matmul(
                    st[:, qs:512], kt[:, h, kb * 128:(kb + 1) * 128],
                    qt[:, h, qg * 512 + qs:(qg + 1) * 512],
                    start=True, stop=(kk < 0))
                if kk >= 0:
                    nc.tensor.matmul(
                        st[:, qs:qs + 128], maskL, maskU,
                        start=False, stop=True)
                pend.append((kb, qs, st))
                if len(pend) > DEPTH:
                    flush_one()
            while pend:
                flush_one()
            return {"b": b, "qg": qg, "h": h, "den_f": den_f, "yt_ps": yt_ps,
                    "nkb": nkb}

        def emit_epiA(u):
            """Denominator: one ones-matmul for the partition reduction of
            the DVE-accumulated partial sums, then fast reciprocal. Emitted
            one unit after u's matmuls so it overlaps the next unit's PE
            work."""
            den16 = dnp.tile([128, 512], f16, tag="den16")
            nc.vector.tensor_copy(den16, u["den_f"])
            den_row = psA.tile([1, 512], f32, tag="psA")
            nc.tensor.matmul(den_row, ones_col, den16, start=True, stop=True)
            rec_sb = dnp.tile([1, 512], f32, tag="rec")
            nc.vector.reciprocal_approx_fast(rec_sb, den_row[0:1, :])
            rec16 = dnp.tile([1, 512], f16, tag="rec16")
            nc.scalar.copy(rec16, rec_sb)
            u["rec16"] = rec16

        def emit_epiB(u, yt):
            """Broadcast 1/den across partitions (PE) and normalize y^T.
            Emitted two units after u's matmuls: the reciprocal has had a
            full unit of slack, so the PE does not stall on the DVE chain."""
            r_ps = psA.tile([128, 512], f32, tag="psA")
            nc.tensor.matmul(r_ps, ones_row, u["rec16"], start=True, stop=True)
            r_sb = rp.tile([128, 512], f32, tag="rsb")
            nc.scalar.copy(r_sb, r_ps)
            nc.vector.tensor_mul(yt[:, u["h"], :], u["yt_ps"], r_sb)

        def emit_proj(b, qg, yt):
            for tt in range(4):
                for co in range(4):
                    o_ps = psA.tile([128, 512], f32, tag="psA")
                    for jh in range(HPC):
                        nc.tensor.matmul(
                            o_ps, yt[:, jh, tt * 128:(tt + 1) * 128],
                            wp_sb[:, jh, co * 512:(co + 1) * 512],
                            start=(jh == 0), stop=(jh == HPC - 1))
                    o_sb = op.tile([128, 512], f16, tag="osb")
                    # alternate PSUM evacuation between DVE and ACT so
                    # neither becomes the PSUM ring's bottleneck
                    if (tt * 4 + co) % 2 == 0:
                        nc.vector.tensor_copy(o_sb, o_ps)
                    else:
                        nc.scalar.copy(o_sb, o_ps)
                    r0 = b * T + qg * 512 + tt * 128
                    nc.sync.dma_start(
                        out[r0:r0 + 128, co * 512:(co + 1) * 512], o_sb)

        def alloc_qkv_tiles():
            qt = qkvp.tile([128, HPC, T], f16, tag="qt")
            kt = qkvp.tile([128, HPC, T], f16, tag="kt")
            vt = qkvp.tile([128, HPC, T], f16, tag="vt")
            v = qkvp.tile([128, T // 128, HPC * HD], f16, tag="v")
            return (qt, kt, vt, v)

        # Pipeline: QKV chunks of batch b+1 interleave into batch b's
        # attention stream, with each chunk's xt DMA issued one chunk ahead
        # so the PE never waits on an in-flight transfer; softmax epilogues
        # trail their unit by 1 (epiA) and 2 (epiB) units so the PE never
        # waits on DVE/ACT results.
        chunk_after = {}
        _seq = [(b, t) for b in range(B) for t in range(NTCH)]
        for _i, _c in enumerate(_seq[:-1]):
            chunk_after[_c] = _seq[_i + 1]

        tiles = alloc_qkv_tiles()
        xt_pend = {}
        # startup: interleave weight-chunk and first-xt-chunk DMA issue so
        # the first matmul waits on two small transfers, not all of them
        xt00 = xtp.tile([128, NCC, 512], f16, tag="xt", name="xt00")
        xt_pend[(0, 0)] = xt00
        for cc in range(NCC):
            nc.sync.dma_start(w_sb[:, cc, :, :], wqkv_v[:, cc, :, :])
            nc.sync.dma_start(
                xt00[:, cc, :], xT[cc * 128:(cc + 1) * 128, 0:512])

        def run_chunk(bt):
            if bt in chunk_after:
                nb = chunk_after[bt]
                xt_pend[nb] = qkv_dma(*nb)
            qkv_mm(bt[0], bt[1], xt_pend.pop(bt),
                   tiles if bt[0] == cur_b else nxt)

        cur_b = 0
        nxt = tiles
        run_chunk((0, 0))
        nc.sync.dma_start(wp_sb, wproj_v)
        for tch in range(1, NTCH):
            run_chunk((0, tch))
        prevA = None   # unit awaiting epiA
        prevB = None   # unit awaiting epiB
        yts = {}       # (b, qg) -> yt tile
        for b in range(B):
            cur_b = b
            nxt = alloc_qkv_tiles() if b + 1 < B else None
            for qg in range(4):
                for h in range(HPC):
                    if h == 0 and nxt is not None:
                        run_chunk((b + 1, qg))
                    u = emit_attn_mm(b, qg, h, tiles)
                    if h == 0:
                        yts[(b, qg)] = ytp.tile(
                            [128, HPC, 512], f16, tag="yt", name=f"yt{b}{qg}")
                    if prevA is not None:
                        emit_epiA(prevA)
                    if prevB is not None:
                        emit_epiB(prevB, yts[(prevB["b"], prevB["qg"])])
                        if prevB["h"] == 1:
                            emit_proj(prevB["b"], prevB["qg"],
                                      yts.pop((prevB["b"], prevB["qg"])))
                    prevB = prevA
                    prevA = u
            tiles = nxt
        # drain the epilogue pipeline
        emit_epiA(prevA)
        emit_epiB(prevB, yts[(prevB["b"], prevB["qg"])])
        if prevB["h"] == 1:
            emit_proj(prevB["b"], prevB["qg"],
                      yts.pop((prevB["b"], prevB["qg"])))
        emit_epiB(prevA, yts[(prevA["b"], prevA["qg"])])
        if prevA["h"] == 1:
            emit_proj(prevA["b"], prevA["qg"],
                      yts.pop((prevA["b"], prevA["qg"])))

    nc.compile()
    return nc


def _get_nc():
    if "nc" not in _CACHE:
        _CACHE["nc"] = _build_nc()
    return _CACHE["nc"]


def _make_in_maps(x2d, Wqkv, Wproj):
    xT = np.ascontiguousarray(x2d.T).astype(np.float16)  # [C, B*T]
    in_maps = []
    for c in range(N_CORES):
        h0 = c * HPC
        cols = []
        for part in range(3):  # q, k, v blocks of Wqkv columns
            for h in range(HPC):
                j0 = part * C + (h0 + h) * HD
                cols.append(Wqkv[:, j0:j0 + HD])
        wq = np.ascontiguousarray(np.concatenate(cols, axis=1)).astype(np.float16)
        wp = np.ascontiguousarray(
            Wproj[h0 * HD:(h0 + HPC) * HD, :]).astype(np.float16)
        in_maps.append({"xt": xT, "wqkv": wq, "wproj": wp})
    return in_maps


def run_shards(in_maps, trace=False):
    from concourse.bass_utils import run_bass_kernel_spmd
    nc = _get_nc()
    last_err = None
    for _attempt in range(3):
        try:
            return run_bass_kernel_spmd(
                nc, in_maps, core_ids=list(range(N_CORES)), trace=trace)
        except Exception as e:  # transient NRT device errors — retry
            last_err = e
            if "UNAVAILABLE" not in str(e) and "UNRECOVERABLE" not in str(e):
                raise
    raise last_err


def kernel(x, Wqkv, Wproj):
    x = np.asarray(x, dtype=np.float32)
    Wqkv = np.asarray(Wqkv, dtype=np.float32)
    Wproj = np.asarray(Wproj, dtype=np.float32)
    x2d = np.ascontiguousarray(x.reshape(B * T, C))

    in_maps = _make_in_maps(x2d, Wqkv, Wproj)
    res = run_shards(in_maps)

    acc = res.results[0]["out"].astype(np.float32)
    for c in range(1, N_CORES):
        acc += res.results[c]["out"].astype(np.float32)
    return acc.reshape(B, T, C)


# revision 17
# speedup vs baseline: 1.3485x; 1.0203x over previous
"""Causal self-attention (B=4, T=2048, C=2048, H=16) on 8 trn2 NeuronCores.

Sharding: tensor-parallel over heads — 2 heads per core. Every core gets the
full (pre-transposed) activation xT, its 2 heads' slice of Wqkv columns and
Wproj rows, computes a full [B*T, C] partial output (fp16), and the host sums
the 8 partials (the "all-reduce after output projection" done host-side).

Per-core dataflow (all matmuls fp16 on PE):
  xT tiles --DMA--> QKV proj -> Q^T,K^T [d,t] + V [t,d]
  S = Q^T.T @ K^T chunks (PSUM f32) -> +causal mask -> exp (ACT) -> P (fp16)
  y^T = sum_k V_k^T-block @ P^T-block (PSUM f32, accumulated over k-blocks)
  softmax denominator: DVE reduce of P over k-blocks -> one ones-matmul ->
  reciprocal_approx_fast -> PE row-broadcast -> DVE normalize.
  The den/rec/normalize epilogue is software-pipelined two attention units
  deep so the in-order PE queue never waits on the DVE/ACT chain (a PE stall
  also drops the PE to its half-speed p-state for ~3us).
  out_partial = y^T.T @ Wproj-rows (accumulate 2 head-chunks) -> fp16 -> DMA
"""
import numpy as np

B, T, C = 4, 2048, 2048
H, HD = 16, 128
N_CORES = 8
HPC = H // N_CORES          # heads per core = 2
SCALE = float(1.0 / np.sqrt(HD))
NEG = -1e9

_CACHE = {}


def _build_nc():
    import concourse.bass as bass
    from concourse import bacc
    import concourse.tile as tile
    import concourse.mybir as mybir
    from concourse.masks import make_identity
    from contextlib import ExitStack

    f32 = mybir.dt.float32
    f16 = mybir.dt.float16
    Exp = mybir.ActivationFunctionType.Exp
    AXX = mybir.AxisListType.X
    Add = mybir.AluOpType.add

    nc = bacc.Bacc("TRN2", target_bir_lowering=False, debug=False,
                   enable_asserts=True, num_devices=N_CORES)

    # Inputs (per-core shards prepared on host)
    xT = nc.dram_tensor("xt", [C, B * T], f16, kind="ExternalInput").ap()
    wqkv = nc.dram_tensor("wqkv", [C, 6 * HD], f16, kind="ExternalInput").ap()
    wproj = nc.dram_tensor("wproj", [HPC * HD, C], f16, kind="ExternalInput").ap()
    out = nc.dram_tensor("out", [B * T, C], f16, kind="ExternalOutput").ap()

    # DRAM views: c-chunked weights
    wqkv_v = wqkv.rearrange("(cc p) (jj d) -> p cc jj d", p=128, d=HD)  # [128,16,6,128]
    wproj_v = wproj.rearrange("(jh p) c -> p jh c", p=128)              # [128,2,2048]

    NCC = C // 128        # 16 contraction chunks
    NTCH = T // 512       # 4 t-chunks per batch

    with tile.TileContext(nc) as tc, ExitStack() as ctx:
        const = ctx.enter_context(tc.tile_pool(name="const", bufs=1))
        wpool = ctx.enter_context(tc.tile_pool(name="w", bufs=1))
        xtp = ctx.enter_context(tc.tile_pool(name="xt", bufs=2))
        qkvp = ctx.enter_context(tc.tile_pool(name="qkv", bufs=2))

        dnp = ctx.enter_context(tc.tile_pool(name="dn", bufs=2))
        rp = ctx.enter_context(tc.tile_pool(name="r", bufs=2))
        ptp = ctx.enter_context(tc.tile_pool(name="pt", bufs=2))
        ytp = ctx.enter_context(tc.tile_pool(name="yt", bufs=2))
        op = ctx.enter_context(tc.tile_pool(name="o", bufs=6))
        psA = ctx.enter_context(tc.tile_pool(name="psA", bufs=4, space="PSUM"))
        psV = ctx.enter_context(tc.tile_pool(name="psV", bufs=3, space="PSUM"))
        psT = ctx.enter_context(tc.tile_pool(name="psT", bufs=1, space="PSUM"))

        ident_f = const.tile([128, 128], f32)
        make_identity(nc, ident_f)
        ident_h = const.tile([128, 128], f16)
        nc.scalar.copy(ident_h, ident_f)
        # causal mask as a PE accumulation: st[k,q] += -60000 * (k > q).
        # maskL = -60000*I (stationary), maskU[c,q] = 1 where c > q (moving);
        # keeps the mask on the Tensor engine - no DVE hop in the S->exp chain
        maskL = const.tile([128, 128], f16)
        nc.scalar.mul(maskL, ident_f, -60000.0)
        mU32 = const.tile([128, 128], f32)
        nc.gpsimd.memset(mU32, 1.0)
        nc.gpsimd.affine_select(
            out=mU32, in_=mU32, compare_op=mybir.AluOpType.is_ge, fill=0.0,
            base=-1, pattern=[[-1, 128]], channel_multiplier=1)
        maskU = const.tile([128, 128], f16)
        nc.scalar.copy(maskU, mU32)
        ones_col = const.tile([128, 1], f16)
        nc.vector.memset(ones_col, 1.0)
        ones_row = const.tile([1, 128], f16)
        nc.vector.memset(ones_row, 1.0)

        w_sb = wpool.tile([128, NCC, 6, HD], f16)
        wp_sb = wpool.tile([128, 2, C], f16)

        def qkv_dma(b, tch):
            t0 = b * T + tch * 512
            xt_t = xtp.tile([128, NCC, 512], f16, tag="xt",
                            name=f"xt{b}{tch}")
            for cc in range(NCC):
                nc.sync.dma_start(
                    xt_t[:, cc, :], xT[cc * 128:(cc + 1) * 128, t0:t0 + 512])
            return xt_t

        def qkv_mm(b, tch, xt_t, qkv_tiles):
            qt, kt, vt, v = qkv_tiles
            for jj in range(6):  # q_h0, q_h1, k_h0, k_h1, v_h0, v_h1
                qk_ps = psA.tile([128, 512], f32, tag="psA")
                for cc in range(NCC):
                    nc.tensor.matmul(qk_ps, w_sb[:, cc, jj, :], xt_t[:, cc, :],
                                     start=(cc == 0), stop=(cc == NCC - 1))
                dst = (qt, qt, kt, kt, vt, vt)[jj]
                # DVE, not ACT: keeps the scalar engine free for the exp
                # stream that gates the in-flight PV matmuls
                nc.vector.tensor_copy(
                    dst[:, jj % 2, tch * 512:(tch + 1) * 512], qk_ps)
            # transpose this chunk's V^T slice -> V [t, d]
            for hh in range(HPC):
                for tb in range(4):
                    tg = tch * 4 + tb
                    vp = psT.tile([128, 128], f16, tag="psT")
                    nc.tensor.transpose(
                        vp, vt[:, hh, tg * 128:(tg + 1) * 128], ident_h)
                    nc.vector.tensor_copy(v[:, tg, hh * HD:(hh + 1) * HD], vp)

        # Global S->exp->PV pipeline, 2 S-blocks deep ACROSS unit boundaries:
        # a unit's tail PV matmuls are covered by the next unit's (or the
        # next QKV chunk's) S matmuls, so the exp latency never exposes the
        # in-order PE queue.
        PIPE = []

        def pipe_flush():
            kb, qs, st, pt_sb, den_f, yt_ps, v_ap, nkb = PIPE.pop(0)
            nc.scalar.activation(
                pt_sb[:, kb, qs:512], st[:, qs:512], Exp, scale=SCALE)
            # running denominator partial sum (per k-partition) on DVE,
            # chasing the exp stream
            if kb == 0:
                nc.vector.tensor_copy(den_f, pt_sb[:, 0, :])
            else:
                nc.vector.tensor_add(
                    den_f[:, qs:512], den_f[:, qs:512], pt_sb[:, kb, qs:512])
            nc.tensor.matmul(
                yt_ps[:, qs:512], v_ap, pt_sb[:, kb, qs:512],
                start=(kb == 0), stop=(kb == nkb - 1))

        def pipe_push(entry):
            PIPE.append(entry)
            if len(PIPE) > 2:
                pipe_flush()

        def pipe_drain():
            while PIPE:
                pipe_flush()

        def emit_attn_mm(b, qg, h, qkv_tiles):
            """S matmuls + exp + PV accumulation for one (batch, q-group,
            head) unit. The softmax epilogue is deferred (see emit_epi*)."""
            qt, kt, vt, v = qkv_tiles
            pt_sb = ptp.tile([128, T // 128, 512], f16, tag="pt")
            den_f = dnp.tile([128, 512], f32, tag="den")
            yt_ps = psV.tile([128, 512], f32, tag="psV")
            nkb = 4 * qg + 4
            for kb in range(nkb):
                kk = kb - 4 * qg
                qs = max(0, kk) * 128
                st = psA.tile([128, 512], f32, tag="psA")
                nc.tensor.matmul(
                    st[:, qs:512], kt[:, h, kb * 128:(kb + 1) * 128],
                    qt[:, h, qg * 512 + qs:(qg + 1) * 512],
                    start=True, stop=(kk < 0))
                if kk >= 0:
                    nc.tensor.matmul(
                        st[:, qs:qs + 128], maskL, maskU,
                        start=False, stop=True)
                pipe_push((kb, qs, st, pt_sb, den_f, yt_ps,
                           v[:, kb, h * HD:(h + 1) * HD], nkb))
            return {"b": b, "qg": qg, "h": h, "den_f": den_f, "yt_ps": yt_ps,
                    "nkb": nkb}

        def emit_epiA(u):
            """Denominator: one ones-matmul for the partition reduction of
            the DVE-accumulated partial sums, then fast reciprocal. Emitted
            one unit after u's matmuls so it overlaps the next unit's PE
            work."""
            den16 = dnp.tile([128, 512], f16, tag="den16")
            nc.vector.tensor_copy(den16, u["den_f"])
            den_row = psA.tile([1, 512], f32, tag="psA")
            nc.tensor.matmul(den_row, ones_col, den16, start=True, stop=True)
            rec_sb = dnp.tile([1, 512], f32, tag="rec")
            nc.vector.reciprocal_approx_fast(rec_sb, den_row[0:1, :])
            rec16 = dnp.tile([1, 512], f16, tag="rec16")
            nc.scalar.copy(rec16, rec_sb)
            u["rec16"] = rec16

        def emit_epiB(u, yt):
            """Broadcast 1/den across partitions (PE) and normalize y^T.
            Emitted two units after u's matmuls: the reciprocal has had a
            full unit of slack, so the PE does not stall on the DVE chain."""
            r_ps = psA.tile([128, 512], f32, tag="psA")
            nc.tensor.matmul(r_ps, ones_row, u["rec16"], start=True, stop=True)
            r_sb = rp.tile([128, 512], f32, tag="rsb")
            nc.scalar.copy(r_sb, r_ps)
            nc.vector.tensor_mul(yt[:, u["h"], :], u["yt_ps"], r_sb)

        def emit_proj(b, qg, yt):
            for tt in range(4):
                for co in range(4):
                    o_ps = psA.tile([128, 512], f32, tag="psA")
                    for jh in range(HPC):
                        nc.tensor.matmul(
                            o_ps, yt[:, jh, tt * 128:(tt + 1) * 128],
                            wp_sb[:, jh, co * 512:(co + 1) * 512],
                            start=(jh == 0), stop=(jh == HPC - 1))
                    o_sb = op.tile([128, 512], f16, tag="osb")
                    # alternate PSUM evacuation between DVE and ACT so
                    # neither becomes the PSUM ring's bottleneck
                    if (tt * 4 + co) % 2 == 0:
                        nc.vector.tensor_copy(o_sb, o_ps)
                    else:
                        nc.scalar.copy(o_sb, o_ps)
                    r0 = b * T + qg * 512 + tt * 128
                    nc.sync.dma_start(
                        out[r0:r0 + 128, co * 512:(co + 1) * 512], o_sb)

        def alloc_qkv_tiles():
            qt = qkvp.tile([128, HPC, T], f16, tag="qt")
            kt = qkvp.tile([128, HPC, T], f16, tag="kt")
            vt = qkvp.tile([128, HPC, T], f16, tag="vt")
            v = qkvp.tile([128, T // 128, HPC * HD], f16, tag="v")
            return (qt, kt, vt, v)

        # Pipeline: QKV chunks of batch b+1 interleave into batch b's
        # attention stream, with each chunk's xt DMA issued one chunk ahead
        # so the PE never waits on an in-flight transfer; softmax epilogues
        # trail their unit by 1 (epiA) and 2 (epiB) units so the PE never
        # waits on DVE/ACT results.
        chunk_after = {}
        _seq = [(b, t) for b in range(B) for t in range(NTCH)]
        for _i, _c in enumerate(_seq[:-1]):
            chunk_after[_c] = _seq[_i + 1]

        tiles = alloc_qkv_tiles()
        xt_pend = {}
        # startup: interleave weight-chunk and first-xt-chunk DMA issue so
        # the first matmul waits on two small transfers, not all of them
        xt00 = xtp.tile([128, NCC, 512], f16, tag="xt", name="xt00")
        xt_pend[(0, 0)] = xt00
        for cc in range(NCC):
            nc.sync.dma_start(w_sb[:, cc, :, :], wqkv_v[:, cc, :, :])
            nc.sync.dma_start(
                xt00[:, cc, :], xT[cc * 128:(cc + 1) * 128, 0:512])

        def run_chunk(bt):
            if bt in chunk_after:
                nb = chunk_after[bt]
                xt_pend[nb] = qkv_dma(*nb)
            qkv_mm(bt[0], bt[1], xt_pend.pop(bt),
                   tiles if bt[0] == cur_b else nxt)

        cur_b = 0
        nxt = tiles
        run_chunk((0, 0))
        nc.sync.dma_start(wp_sb, wproj_v)
        for tch in range(1, NTCH):
            run_chunk((0, tch))
        prevA = None   # unit awaiting epiA
        prevB = None   # unit awaiting epiB
        yts = {}       # (b, qg) -> yt tile
        for b in range(B):
            cur_b = b
            nxt = alloc_qkv_tiles() if b + 1 < B else None
            for qg in range(4):
                for h in range(HPC):
                    if h == 0 and nxt is not None:
                        run_chunk((b + 1, qg))
                    u = emit_attn_mm(b, qg, h, tiles)
                    if h == 0:
                        yts[(b, qg)] = ytp.tile(
                            [128, HPC, 512], f16, tag="yt", name=f"yt{b}{qg}")
                    if prevA is not None:
                        emit_epiA(prevA)
                    if prevB is not None:
                        emit_epiB(prevB, yts[(prevB["b"], prevB["qg"])])
                        if prevB["h"] == 1:
                            emit_proj(prevB["b"], prevB["qg"],
                                      yts.pop((prevB["b"], prevB["qg"])))
                    prevB = prevA
                    prevA = u
            tiles = nxt
        # drain the epilogue pipeline
        pipe_drain()
        emit_epiA(prevA)
        emit_epiB(prevB, yts[(prevB["b"], prevB["qg"])])
        if prevB["h"] == 1:
            emit_proj(prevB["b"], prevB["qg"],
                      yts.pop((prevB["b"], prevB["qg"])))
        emit_epiB(prevA, yts[(prevA["b"], prevA["qg"])])
        if prevA["h"] == 1:
            emit_proj(prevA["b"], prevA["qg"],
                      yts.pop((prevA["b"], prevA["qg"])))

    nc.compile()
    return nc


def _get_nc():
    if "nc" not in _CACHE:
        _CACHE["nc"] = _build_nc()
    return _CACHE["nc"]


def _make_in_maps(x2d, Wqkv, Wproj):
    xT = np.ascontiguousarray(x2d.T).astype(np.float16)  # [C, B*T]
    in_maps = []
    for c in range(N_CORES):
        h0 = c * HPC
        cols = []
        for part in range(3):  # q, k, v blocks of Wqkv columns
            for h in range(HPC):
                j0 = part * C + (h0 + h) * HD
                cols.append(Wqkv[:, j0:j0 + HD])
        wq = np.ascontiguousarray(np.concatenate(cols, axis=1)).astype(np.float16)
        wp = np.ascontiguousarray(
            Wproj[h0 * HD:(h0 + HPC) * HD, :]).astype(np.float16)
        in_maps.append({"xt": xT, "wqkv": wq, "wproj": wp})
    return in_maps


def run_shards(in_maps, trace=False):
    from concourse.bass_utils import run_bass_kernel_spmd
    nc = _get_nc()
    last_err = None
    for _attempt in range(3):
        try:
            return run_bass_kernel_spmd(
                nc, in_maps, core_ids=list(range(N_CORES)), trace=trace)
        except Exception as e:  # transient NRT device errors — retry
            last_err = e
            if "UNAVAILABLE" not in str(e) and "UNRECOVERABLE" not in str(e):
                raise
    raise last_err


def kernel(x, Wqkv, Wproj):
    x = np.asarray(x, dtype=np.float32)
    Wqkv = np.asarray(Wqkv, dtype=np.float32)
    Wproj = np.asarray(Wproj, dtype=np.float32)
    x2d = np.ascontiguousarray(x.reshape(B * T, C))

    in_maps = _make_in_maps(x2d, Wqkv, Wproj)
    res = run_shards(in_maps)

    acc = res.results[0]["out"].astype(np.float32)
    for c in range(1, N_CORES):
        acc += res.results[c]["out"].astype(np.float32)
    return acc.reshape(B, T, C)


# revision 20
# speedup vs baseline: 1.4141x; 1.0486x over previous
"""Causal self-attention (B=4, T=2048, C=2048, H=16) on 8 trn2 NeuronCores.

Sharding: tensor-parallel over heads — 2 heads per core. Every core gets the
full (pre-transposed) activation xT, its 2 heads' slice of Wqkv columns and
Wproj rows, computes a full [B*T, C] partial output (fp16), and the host sums
the 8 partials (the "all-reduce after output projection" done host-side).

Per-core dataflow (all matmuls fp16 on PE):
  xT tiles --DMA--> QKV proj -> Q^T,K^T [d,t] + V [t,d]
  S = Q^T.T @ K^T chunks (PSUM f32) -> +causal mask -> exp (ACT) -> P (fp16)
  y^T = sum_k V_k^T-block @ P^T-block (PSUM f32, accumulated over k-blocks)
  softmax denominator: DVE reduce of P over k-blocks -> one ones-matmul ->
  reciprocal_approx_fast -> PE row-broadcast -> DVE normalize.
  The den/rec/normalize epilogue is software-pipelined two attention units
  deep so the in-order PE queue never waits on the DVE/ACT chain (a PE stall
  also drops the PE to its half-speed p-state for ~3us).
  out_partial = y^T.T @ Wproj-rows (accumulate 2 head-chunks) -> fp16 -> DMA
"""
import numpy as np

B, T, C = 4, 2048, 2048
H, HD = 16, 128
N_CORES = 8
HPC = H // N_CORES          # heads per core = 2
SCALE = float(1.0 / np.sqrt(HD))
NEG = -1e9

_CACHE = {}


def _build_nc():
    import concourse.bass as bass
    from concourse import bacc
    import concourse.tile as tile
    import concourse.mybir as mybir
    from concourse.masks import make_identity
    from contextlib import ExitStack

    f32 = mybir.dt.float32
    f16 = mybir.dt.float16
    Exp = mybir.ActivationFunctionType.Exp
    AXX = mybir.AxisListType.X
    Add = mybir.AluOpType.add

    nc = bacc.Bacc("TRN2", target_bir_lowering=False, debug=False,
                   enable_asserts=True, num_devices=N_CORES)

    # Inputs (per-core shards prepared on host)
    xT = nc.dram_tensor("xt", [C, B * T], f16, kind="ExternalInput").ap()
    wqkv = nc.dram_tensor("wqkv", [C, 6 * HD], f16, kind="ExternalInput").ap()
    wproj = nc.dram_tensor("wproj", [HPC * HD, C], f16, kind="ExternalInput").ap()
    out = nc.dram_tensor("out", [B * T, C], f16, kind="ExternalOutput").ap()

    # DRAM views: c-chunked weights
    wqkv_v = wqkv.rearrange("(cc p) (jj d) -> p cc jj d", p=128, d=HD)  # [128,16,6,128]
    wproj_v = wproj.rearrange("(jh p) c -> p jh c", p=128)              # [128,2,2048]

    NCC = C // 128        # 16 contraction chunks
    NTCH = T // 512       # 4 t-chunks per batch

    with tile.TileContext(nc) as tc, ExitStack() as ctx:
        const = ctx.enter_context(tc.tile_pool(name="const", bufs=1))
        wpool = ctx.enter_context(tc.tile_pool(name="w", bufs=1))
        xtp = ctx.enter_context(tc.tile_pool(name="xt", bufs=2))
        qkvp = ctx.enter_context(tc.tile_pool(name="qkv", bufs=2))

        dnp = ctx.enter_context(tc.tile_pool(name="dn", bufs=2))
        rp = ctx.enter_context(tc.tile_pool(name="r", bufs=2))
        ptp = ctx.enter_context(tc.tile_pool(name="pt", bufs=2))
        ytp = ctx.enter_context(tc.tile_pool(name="yt", bufs=2))
        op = ctx.enter_context(tc.tile_pool(name="o", bufs=10))
        psA = ctx.enter_context(tc.tile_pool(name="psA", bufs=4, space="PSUM"))
        psV = ctx.enter_context(tc.tile_pool(name="psV", bufs=3, space="PSUM"))
        psT = ctx.enter_context(tc.tile_pool(name="psT", bufs=1, space="PSUM"))

        ident_f = const.tile([128, 128], f32)
        make_identity(nc, ident_f)
        ident_h = const.tile([128, 128], f16)
        nc.scalar.copy(ident_h, ident_f)
        # causal mask as a PE accumulation: st[k,q] += -60000 * (k > q).
        # maskL = -60000*I (stationary), maskU[c,q] = 1 where c > q (moving);
        # keeps the mask on the Tensor engine - no DVE hop in the S->exp chain
        maskL = const.tile([128, 128], f16)
        nc.scalar.mul(maskL, ident_f, -60000.0)
        mU32 = const.tile([128, 128], f32)
        nc.gpsimd.memset(mU32, 1.0)
        nc.gpsimd.affine_select(
            out=mU32, in_=mU32, compare_op=mybir.AluOpType.is_ge, fill=0.0,
            base=-1, pattern=[[-1, 128]], channel_multiplier=1)
        maskU = const.tile([128, 128], f16)
        nc.scalar.copy(maskU, mU32)
        ones_col = const.tile([128, 1], f16)
        nc.vector.memset(ones_col, 1.0)
        ones_row = const.tile([1, 128], f16)
        nc.vector.memset(ones_row, 1.0)

        w_sb = wpool.tile([128, NCC, 6, HD], f16)
        wp_sb = wpool.tile([128, 2, C], f16)

        def qkv_dma(b, tch):
            t0 = b * T + tch * 512
            xt_t = xtp.tile([128, NCC, 512], f16, tag="xt",
                            name=f"xt{b}{tch}")
            for cc in range(NCC):
                nc.sync.dma_start(
                    xt_t[:, cc, :], xT[cc * 128:(cc + 1) * 128, t0:t0 + 512])
            return xt_t

        def qkv_mm(b, tch, xt_t, qkv_tiles):
            qt, kt, vt, v = qkv_tiles
            for jj in range(6):  # q_h0, q_h1, k_h0, k_h1, v_h0, v_h1
                qk_ps = psA.tile([128, 512], f32, tag="psA")
                for cc in range(NCC):
                    nc.tensor.matmul(qk_ps, w_sb[:, cc, jj, :], xt_t[:, cc, :],
                                     start=(cc == 0), stop=(cc == NCC - 1))
                dst = (qt, qt, kt, kt, vt, vt)[jj]
                # DVE, not ACT: keeps the scalar engine free for the exp
                # stream that gates the in-flight PV matmuls
                nc.vector.tensor_copy(
                    dst[:, jj % 2, tch * 512:(tch + 1) * 512], qk_ps)
            # transpose this chunk's V^T slice -> V [t, d]
            for hh in range(HPC):
                for tb in range(4):
                    tg = tch * 4 + tb
                    vp = psT.tile([128, 128], f16, tag="psT")
                    nc.tensor.transpose(
                        vp, vt[:, hh, tg * 128:(tg + 1) * 128], ident_h)
                    nc.vector.tensor_copy(v[:, tg, hh * HD:(hh + 1) * HD], vp)

        # Global S->exp->PV pipeline, 2 S-blocks deep ACROSS unit boundaries:
        # a unit's tail PV matmuls are covered by the next unit's (or the
        # next QKV chunk's) S matmuls, so the exp latency never exposes the
        # in-order PE queue.
        PIPE = []

        def pipe_flush():
            kb, qs, st, pt_sb, den_f, yt_ps, v_ap, nkb = PIPE.pop(0)
            nc.scalar.activation(
                pt_sb[:, kb, qs:512], st[:, qs:512], Exp, scale=SCALE)
            # running denominator partial sum (per k-partition) on DVE,
            # chasing the exp stream
            if kb == 0:
                nc.vector.tensor_copy(den_f, pt_sb[:, 0, :])
            else:
                nc.vector.tensor_add(
                    den_f[:, qs:512], den_f[:, qs:512], pt_sb[:, kb, qs:512])
            nc.tensor.matmul(
                yt_ps[:, qs:512], v_ap, pt_sb[:, kb, qs:512],
                start=(kb == 0), stop=(kb == nkb - 1))

        def pipe_push(entry):
            PIPE.append(entry)
            if len(PIPE) > 2:
                pipe_flush()

        def pipe_drain():
            while PIPE:
                pipe_flush()

        def emit_attn_mm(b, qg, h, qkv_tiles):
            """S matmuls + exp + PV accumulation for one (batch, q-group,
            head) unit. The softmax epilogue is deferred (see emit_epi*)."""
            qt, kt, vt, v = qkv_tiles
            pt_sb = ptp.tile([128, T // 128, 512], f16, tag="pt")
            den_f = dnp.tile([128, 512], f32, tag="den")
            yt_ps = psV.tile([128, 512], f32, tag="psV")
            nkb = 4 * qg + 4
            for kb in range(nkb):
                kk = kb - 4 * qg
                qs = max(0, kk) * 128
                st = psA.tile([128, 512], f32, tag="psA")
                nc.tensor.matmul(
                    st[:, qs:512], kt[:, h, kb * 128:(kb + 1) * 128],
                    qt[:, h, qg * 512 + qs:(qg + 1) * 512],
                    start=True, stop=(kk < 0))
                if kk >= 0:
                    nc.tensor.matmul(
                        st[:, qs:qs + 128], maskL, maskU,
                        start=False, stop=True)
                pipe_push((kb, qs, st, pt_sb, den_f, yt_ps,
                           v[:, kb, h * HD:(h + 1) * HD], nkb))
            return {"b": b, "qg": qg, "h": h, "den_f": den_f, "yt_ps": yt_ps,
                    "nkb": nkb}

        def emit_epiA(u):
            """Denominator: one ones-matmul for the partition reduction of
            the DVE-accumulated partial sums, then fast reciprocal. Emitted
            one unit after u's matmuls so it overlaps the next unit's PE
            work."""
            den16 = dnp.tile([128, 512], f16, tag="den16")
            nc.vector.tensor_copy(den16, u["den_f"])
            den_row = psA.tile([1, 512], f32, tag="psA")
            nc.tensor.matmul(den_row, ones_col, den16, start=True, stop=True)
            rec_sb = dnp.tile([1, 512], f32, tag="rec")
            nc.vector.reciprocal_approx_fast(rec_sb, den_row[0:1, :])
            rec16 = dnp.tile([1, 512], f16, tag="rec16")
            nc.scalar.copy(rec16, rec_sb)
            u["rec16"] = rec16

        def emit_epiB(u, yt):
            """Broadcast 1/den across partitions (PE) and normalize y^T.
            Emitted two units after u's matmuls: the reciprocal has had a
            full unit of slack, so the PE does not stall on the DVE chain."""
            r_ps = psA.tile([128, 512], f32, tag="psA")
            nc.tensor.matmul(r_ps, ones_row, u["rec16"], start=True, stop=True)
            r_sb = rp.tile([128, 512], f32, tag="rsb")
            nc.scalar.copy(r_sb, r_ps)
            nc.vector.tensor_mul(yt[:, u["h"], :], u["yt_ps"], r_sb)

        def emit_proj(b, qg, yt):
            for tt in range(4):
                for co in range(4):
                    o_ps = psA.tile([128, 512], f32, tag="psA")
                    for jh in range(HPC):
                        nc.tensor.matmul(
                            o_ps, yt[:, jh, tt * 128:(tt + 1) * 128],
                            wp_sb[:, jh, co * 512:(co + 1) * 512],
                            start=(jh == 0), stop=(jh == HPC - 1))
                    o_sb = op.tile([128, 512], f16, tag="osb")
                    # alternate PSUM evacuation between DVE and ACT so
                    # neither becomes the PSUM ring's bottleneck
                    if (tt * 4 + co) % 2 == 0:
                        nc.vector.tensor_copy(o_sb, o_ps)
                    else:
                        nc.scalar.copy(o_sb, o_ps)
                    r0 = b * T + qg * 512 + tt * 128
                    nc.sync.dma_start(
                        out[r0:r0 + 128, co * 512:(co + 1) * 512], o_sb)

        def alloc_qkv_tiles():
            qt = qkvp.tile([128, HPC, T], f16, tag="qt")
            kt = qkvp.tile([128, HPC, T], f16, tag="kt")
            vt = qkvp.tile([128, HPC, T], f16, tag="vt")
            v = qkvp.tile([128, T // 128, HPC * HD], f16, tag="v")
            return (qt, kt, vt, v)

        # Pipeline: QKV chunks of batch b+1 interleave into batch b's
        # attention stream, with each chunk's xt DMA issued one chunk ahead
        # so the PE never waits on an in-flight transfer; softmax epilogues
        # trail their unit by 1 (epiA) and 2 (epiB) units so the PE never
        # waits on DVE/ACT results.
        chunk_after = {}
        _seq = [(b, t) for b in range(B) for t in range(NTCH)]
        for _i, _c in enumerate(_seq[:-1]):
            chunk_after[_c] = _seq[_i + 1]

        tiles = alloc_qkv_tiles()
        xt_pend = {}
        # startup: interleave weight-chunk and first-xt-chunk DMA issue so
        # the first matmul waits on two small transfers, not all of them
        xt00 = xtp.tile([128, NCC, 512], f16, tag="xt", name="xt00")
        xt_pend[(0, 0)] = xt00
        for cc in range(NCC):
            nc.sync.dma_start(w_sb[:, cc, :, :], wqkv_v[:, cc, :, :])
            nc.sync.dma_start(
                xt00[:, cc, :], xT[cc * 128:(cc + 1) * 128, 0:512])

        def run_chunk(bt):
            if bt in chunk_after:
                nb = chunk_after[bt]
                xt_pend[nb] = qkv_dma(*nb)
            qkv_mm(bt[0], bt[1], xt_pend.pop(bt),
                   tiles if bt[0] == cur_b else nxt)

        cur_b = 0
        nxt = tiles
        run_chunk((0, 0))
        nc.sync.dma_start(wp_sb, wproj_v)
        for tch in range(1, NTCH):
            run_chunk((0, tch))
        prevA = None   # unit awaiting epiA
        prevB = None   # unit awaiting epiB
        yts = {}       # (b, qg) -> yt tile
        for b in range(B):
            cur_b = b
            nxt = alloc_qkv_tiles() if b + 1 < B else None
            for qg in range(4):
                for h in range(HPC):
                    if h == 0 and nxt is not None:
                        run_chunk((b + 1, qg))
                    u = emit_attn_mm(b, qg, h, tiles)
                    if h == 0:
                        yts[(b, qg)] = ytp.tile(
                            [128, HPC, 512], f16, tag="yt", name=f"yt{b}{qg}")
                    if prevA is not None:
                        emit_epiA(prevA)
                    if prevB is not None:
                        emit_epiB(prevB, yts[(prevB["b"], prevB["qg"])])
                        if prevB["h"] == 1:
                            emit_proj(prevB["b"], prevB["qg"],
                                      yts.pop((prevB["b"], prevB["qg"])))
                    prevB = prevA
                    prevA = u
            tiles = nxt
        # drain the epilogue pipeline
        pipe_drain()
        emit_epiA(prevA)
        emit_epiB(prevB, yts[(prevB["b"], prevB["qg"])])
        if prevB["h"] == 1:
            emit_proj(prevB["b"], prevB["qg"],
                      yts.pop((prevB["b"], prevB["qg"])))
        emit_epiB(prevA, yts[(prevA["b"], prevA["qg"])])
        if prevA["h"] == 1:
            emit_proj(prevA["b"], prevA["qg"],
                      yts.pop((prevA["b"], prevA["qg"])))

    nc.compile()
    return nc


def _get_nc():
    if "nc" not in _CACHE:
        _CACHE["nc"] = _build_nc()
    return _CACHE["nc"]


def _make_in_maps(x2d, Wqkv, Wproj):
    xT = np.ascontiguousarray(x2d.T).astype(np.float16)  # [C, B*T]
    in_maps = []
    for c in range(N_CORES):
        h0 = c * HPC
        cols = []
        for part in range(3):  # q, k, v blocks of Wqkv columns
            for h in range(HPC):
                j0 = part * C + (h0 + h) * HD
                cols.append(Wqkv[:, j0:j0 + HD])
        wq = np.ascontiguousarray(np.concatenate(cols, axis=1)).astype(np.float16)
        wp = np.ascontiguousarray(
            Wproj[h0 * HD:(h0 + HPC) * HD, :]).astype(np.float16)
        in_maps.append({"xt": xT, "wqkv": wq, "wproj": wp})
    return in_maps


def run_shards(in_maps, trace=False):
    from concourse.bass_utils import run_bass_kernel_spmd
    nc = _get_nc()
    last_err = None
    for _attempt in range(3):
        try:
            return run_bass_kernel_spmd(
                nc, in_maps, core_ids=list(range(N_CORES)), trace=trace)
        except Exception as e:  # transient NRT device errors — retry
            last_err = e
            if "UNAVAILABLE" not in str(e) and "UNRECOVERABLE" not in str(e):
                raise
    raise last_err


def kernel(x, Wqkv, Wproj):
    x = np.asarray(x, dtype=np.float32)
    Wqkv = np.asarray(Wqkv, dtype=np.float32)
    Wproj = np.asarray(Wproj, dtype=np.float32)
    x2d = np.ascontiguousarray(x.reshape(B * T, C))

    in_maps = _make_in_maps(x2d, Wqkv, Wproj)
    res = run_shards(in_maps)

    acc = res.results[0]["out"].astype(np.float32)
    for c in range(1, N_CORES):
        acc += res.results[c]["out"].astype(np.float32)
    return acc.reshape(B, T, C)
